# revision 23
# baseline (speedup 1.0000x reference)
"""Trainium2 Bass kernel for nn_Attention_50843822850577.

Reference computation (per batch b):
  Q = Wq @ norm(content) + bq ; K = Wk @ norm(style) + bk ; V = Wv @ style + bv
  S = Q^T K  (N x N);  A = softmax(S, axis=-1);  Out = V @ A^T

Sharding: 8 cores = 4 batches x 2 query-halves. Each core gets the full
content/style for its batch (stats need all spatial positions; content is
permuted so the core's query half occupies columns [0, NQ)), computes
Out[:, its-half] and the host scatters halves back together.

v2 structure (phase-1 overlap + PE-offload of softmax bookkeeping):
  - stream Y fully first (V^T matmuls ride the stream), fold K weights,
    then K projection (PE) overlaps the X stream (DMA/DVE only)
  - Q projection, sampled row-max and the G broadcast are emitted per
    512-query chunk so the PE queue never drains
  - E' slab [128, MT, 512] bf16 per chunk: Z = sum E' via DVE slab adds
    (emitted as the slab fills) + one ones-matmul; the old per-m-tile PE
    Z matmuls are gone
  - V tiles and E' in bf16 (validated numerics; fp16 would overflow E')
  - chunk-n normalization (Z matmul, 1/Z, broadcast, U*1/Z, DMA out) is
    interleaved into chunk n+1's S-matmul stream so PE never waits

Numerics (validated in numpy emulation + HW):
  - mean/var stats and all accumulation in fp32
  - normalization folded into the weights: Q = (Wq*inv) @ X_raw + (bq - Wq*inv @ mu)
  - Q/K/S matmuls in fp16 (HW relL2 ~3e-4/matmul)
  - softmax shift G_n = rowmax-over-first-128-keys + 40: the shift cancels
    exactly; sampling margin validated on the reference input distribution
  - E' = exp(S - G) bf16 (range to e^85 ok), V bf16 for the O matmul
  - per-row normalization by Z = sum E' via DVE slab-reduce + ones matmul
"""
import numpy as np

import concourse.bass as bass
import concourse.mybir as mybir
import concourse.tile as tile
from concourse import bacc
from concourse.masks import make_identity
from concourse.bass_utils import run_bass_kernel_spmd

F32 = mybir.dt.float32
F16 = mybir.dt.float16
F32R = mybir.dt.float32r
BF16 = mybir.dt.bfloat16
AX = mybir.AxisListType
ACT = mybir.ActivationFunctionType

EPS = 1e-5
G_OFFSET = 40.0


def build_attention(C=512, NK=4096, NQ=2048, ev_dtype=BF16, hkc=512, depth=5):
    """One-core SPMD program: full attention for one (batch, query-half)."""
    assert C % 128 == 0 and NK % 1024 == 0 and NQ % 512 == 0 and NQ <= NK // 2
    CT = C // 128          # contraction/channel tiles
    MT = NK // 128         # key (m) tiles
    NCH = NQ // 512        # query chunks of 512
    NT = NQ // 128         # query tiles of 128
    HKC = hkc              # n-major streaming chunk width
    NCC = NK // HKC        # streaming chunks per input
    MPC = HKC // 128       # m-tiles per streaming chunk
    ddof_scale = NK / (NK - 1)

    nc = bacc.Bacc("TRN2", target_bir_lowering=False, debug=False)
    xq = nc.dram_tensor("xq", [C, NK], F32, kind="ExternalInput")
    y = nc.dram_tensor("y", [C, NK], F32, kind="ExternalInput")
    wqt = nc.dram_tensor("wqt", [C, C], F32, kind="ExternalInput")
    wkt = nc.dram_tensor("wkt", [C, C], F32, kind="ExternalInput")
    wvt = nc.dram_tensor("wvt", [C, C], F32, kind="ExternalInput")
    bq = nc.dram_tensor("bq", [C], F32, kind="ExternalInput")
    bk = nc.dram_tensor("bk", [C], F32, kind="ExternalInput")
    bv = nc.dram_tensor("bv", [C], F32, kind="ExternalInput")
    o = nc.dram_tensor("o", [C, NQ], F32, kind="ExternalOutput")

    with tile.TileContext(nc) as tc:
      with tc.tile_pool(name="persist", bufs=1) as persist:
        ones32 = persist.tile([1, 128], F32, name="ones32")
        nc.vector.memset(ones32[:], 1.0)
        onesr = persist.tile([128, 1], F32, name="onesr")
        nc.vector.memset(onesr[:], 1.0)
        q16 = persist.tile([128, CT, NQ], F16, name="q16")
        k16 = persist.tile([128, CT, NK], F16, name="k16")
        vt = persist.tile([128, MT, C], ev_dtype, name="vt")
        bg = persist.tile([128, NQ], F32, name="bg")
        ident = persist.tile([128, 128], F32, name="ident")
        make_identity(nc, ident)
        ones16 = persist.tile([1, 128], F16, name="ones16")
        nc.vector.memset(ones16[:], 1.0)

        # ---------------- phase 1: stream + stats + projections ----------
        with (
            tc.tile_pool(name="ph1", bufs=1) as p1,
            tc.tile_pool(name="psA", bufs=3, space="PSUM") as psA,
        ):
            y16 = p1.tile([128, CT, NK], F16, name="y16")
            x16 = p1.tile([128, CT, NQ], F16, name="x16")
            wv16 = p1.tile([128, CT, C], F16, name="wv16")
            wk16 = p1.tile([128, CT, C], F16, name="wk16")
            wq16 = p1.tile([128, CT, C], F16, name="wq16")
            inv_x = p1.tile([128, CT, 1], F32, name="inv_x")
            inv_y = p1.tile([128, CT, 1], F32, name="inv_y")
            mu_x16 = p1.tile([128, CT, 1], F16, name="mu_x16")
            mu_y16 = p1.tile([128, CT, 1], F16, name="mu_y16")
            eps_t = p1.tile([128, 1], F32, name="eps_t")
            nc.vector.memset(eps_t[:], EPS)
            bq_sb = p1.tile([128, CT, 1], F32, name="bq_sb")
            bk_sb = p1.tile([128, CT, 1], F32, name="bk_sb")
            bqp = p1.tile([128, CT, 1], F32, name="bqp")
            bkp = p1.tile([128, CT, 1], F32, name="bkp")
            stats_y = p1.tile([128, CT, 8, 6], F32, name="stats_y")
            stats_x = p1.tile([128, CT, 8, 6], F32, name="stats_x")
            mt_max = p1.tile([128, NT, 1], F32, name="mt_max")
            bgrow = p1.tile([1, NQ], F16, name="bgrow")
            dma_engs = (nc.sync, nc.scalar, nc.gpsimd)

            def fold_stats(stats_t, inv_t, mu16_t, tagp):
                # batched: 4x bn_aggr, then ONE sqrt / reciprocal / copy over
                # [128, CT] so the post-stream serial fold chain is short
                mv4 = p1.tile([128, CT, 2], F32, name=f"mv4_{tagp}", tag="mv")
                for ct in range(CT):
                    nc.vector.bn_aggr(out=mv4[:, ct], in_=stats_t[:, ct])
                std4 = p1.tile([128, CT, 1], F32, name=f"std4_{tagp}", tag="std")
                nc.scalar.activation(out=std4[:], in_=mv4[:, :, 1:2], func=ACT.Sqrt,
                                     bias=eps_t[:], scale=float(ddof_scale))
                nc.vector.reciprocal(out=inv_t[:], in_=std4[:])
                nc.vector.tensor_copy(out=mu16_t[:], in_=mv4[:, :, 0:1])

            def fold_weights(wsrc, wdst, inv_t, eng):
                for ct in range(CT):
                    wraw = p1.tile([128, C], F32, name=f"wraw_{ct}", tag="raw", bufs=2)
                    eng.dma_start(out=wraw[:], in_=wsrc[bass.ts(ct, 128), :])
                    if inv_t is None:
                        nc.vector.tensor_copy(out=wdst[:, ct, :], in_=wraw[:])
                    else:
                        nc.vector.tensor_scalar_mul(wdst[:, ct, :], in0=wraw[:],
                                                    scalar1=inv_t[:, ct, :])

            def fold_bias(wdst, mu16_t, b_sb, bp):
                for ot in range(CT):
                    pb = psA.tile([128, 1], F32, name=f"pb_{ot}", tag="mm")
                    for ct in range(CT):
                        nc.tensor.matmul(pb[:], wdst[:, ct, bass.ts(ot, 128)],
                                         mu16_t[:, ct, :],
                                         start=(ct == 0), stop=(ct == CT - 1))
                    nc.vector.tensor_sub(bp[:, ot, :], in0=b_sb[:, ot, :], in1=pb[:])

            def proj_chunk(w16, src16, bp, dst, j):
                # dst[o, j*512 : +512] = W^T @ src + b  (bias-add on ACT:
                # bias is per output channel = per partition, so the PSUM
                # eviction to fp16 absorbs it and the DVE stays free)
                for ot in range(CT):
                    pq = psA.tile([128, 512], F32, name=f"pq_{ot}_{j}", tag="mm")
                    for ct in range(CT):
                        nc.tensor.matmul(pq[:], w16[:, ct, bass.ts(ot, 128)],
                                         src16[:, ct, bass.ts(j, 512)],
                                         start=(ct == 0), stop=(ct == CT - 1))
                    nc.scalar.activation(out=dst[:, ot, bass.ts(j, 512)], in_=pq[:],
                                         func=ACT.Identity, bias=bp[:, ot, :])

            QW = NK // 4    # column-quarter width: contiguous 4KB DRAM runs

            def stream_quarter(src, raw_tag, stats_t, dst16, dst_cols, h):
                # one DMA per (ct, quarter): [128 rows, QW cols] is 4KB
                # contiguous per DRAM row -- near-peak HBM rate (the n-major
                # rearrange pattern capped at ~205 GB/s with 2KB runs)
                for ct in range(CT):
                    raw = p1.tile([128, QW], F32, name=f"{raw_tag}_{h}_{ct}",
                                  tag=raw_tag, bufs=6 if raw_tag == "rawy" else 4)
                    nc.sync.dma_start(
                        out=raw[:],
                        in_=src[bass.ts(ct, 128), bass.ts(h, QW)])
                    nc.vector.bn_stats(out=stats_t[:, ct, 2 * h, :], in_=raw[:, 0:512])
                    nc.vector.bn_stats(out=stats_t[:, ct, 2 * h + 1, :], in_=raw[:, 512:1024])
                    if h * QW < dst_cols:
                        nc.scalar.copy(out=dst16[:, ct, bass.ts(h, QW)], in_=raw[:])

            def v_quarter(h):
                for mi in range(QW // 128):
                    mt = h * (QW // 128) + mi
                    pv = psA.tile([128, C], F32, name=f"pv_{mt}", tag="mm")
                    for ct in range(CT):
                        nc.tensor.matmul(
                            pv[:], y16[:, ct, bass.ts(mt, 128)], wv16[:, ct, :],
                            start=(ct == 0), stop=(ct == CT - 1))
                    # no +bv here: softmax rows sum to 1, so Out = (Wv Y)A^T + bv
                    # exactly -- bv is added on the host
                    nc.scalar.copy(out=vt[:, mt, :], in_=pv[:])

            # ---- Y stream: quarter-major (all 4 ct tiles of a column
            # quarter land together so V^T of that quarter can run); V is
            # deferred one quarter so ACT emits copies ahead of evictions.
            # The last V quarter is the PE filler for the stats fold. ----
            stream_quarter(y, "rawy", stats_y, y16, NK, 0)
            fold_weights(wvt, wv16, None, nc.gpsimd)
            nc.gpsimd.dma_start(out=bk_sb[:], in_=bk.rearrange("(t p one) -> p t one", p=128, one=1))
            nc.gpsimd.dma_start(out=bq_sb[:], in_=bq.rearrange("(t p one) -> p t one", p=128, one=1))
            for h in range(1, 4):
                stream_quarter(y, "rawy", stats_y, y16, NK, h)
                v_quarter(h - 1)

            fold_stats(stats_y, inv_y, mu_y16, "y")
            fold_weights(wkt, wk16, inv_y, nc.sync)
            v_quarter(3)
            fold_bias(wk16, mu_y16, bk_sb, bkp)

            # ---- X stream (DMA/DVE) interleaved with K projection (PE/ACT).
            # x16 copies are emitted AFTER the K-proj evictions of the same
            # round so they never block the pq PSUM rotation; the last K
            # chunk is the PE filler for the X-stats fold chain. ----
            for h in range(4):
                if h >= 1:
                    proj_chunk(wk16, y16, bkp, k16, 2 * (h - 1))
                    proj_chunk(wk16, y16, bkp, k16, 2 * (h - 1) + 1)
                stream_quarter(xq, "rawx", stats_x, x16, NQ, h)

            fold_stats(stats_x, inv_x, mu_x16, "x")
            fold_weights(wqt, wq16, inv_x, nc.sync)
            proj_chunk(wk16, y16, bkp, k16, 6)
            proj_chunk(wk16, y16, bkp, k16, 7)
            fold_bias(wq16, mu_x16, bq_sb, bqp)

            # ---- Q projection + sampled row-max + G broadcast. Two passes:
            # pss+reduce ride each Q chunk (PE dense), then all transposes,
            # then the per-chunk broadcasts -- no PE<->DVE ping-pong. ----
            for jq in range(NCH):
                proj_chunk(wq16, x16, bqp, q16, jq)
                for nt in range(jq * 4, jq * 4 + 4):
                    pss = psA.tile([128, 128], F32, name=f"pss_{nt}", tag="pss", bufs=2)
                    for ct in range(CT):
                        nc.tensor.matmul(pss[:], q16[:, ct, bass.ts(nt, 128)],
                                         k16[:, ct, 0:128],
                                         start=(ct == 0), stop=(ct == CT - 1))
                    nc.vector.reduce_max(out=mt_max[:, nt, :], in_=pss[:], axis=AX.X)
            for nt in range(NT):
                ps_t = psA.tile([1, 128], F32, name=f"ps_t_{nt}", tag="pst", bufs=2)
                nc.tensor.transpose(ps_t[:], mt_max[:, nt, :], ident[:])
                nc.scalar.activation(out=bgrow[:, bass.ts(nt, 128)], in_=ps_t[:],
                                     func=ACT.Copy, bias=G_OFFSET)
            for jq in range(NCH):
                pbg = psA.tile([128, 512], F32, name=f"pbg_{jq}", tag="mm")
                nc.tensor.matmul(pbg[:], ones16[:], bgrow[:, bass.ts(jq, 512)],
                                 start=True, stop=True)
                nc.vector.tensor_copy(out=bg[:, bass.ts(jq, 512)], in_=pbg[:])

        # ---------------- phase 2: S -> E' -> U, Z -> O -------------------
        with (
            tc.tile_pool(name="ph2", bufs=1) as p2,
            tc.tile_pool(name="psB", bufs=1, space="PSUM") as psB,
        ):
            u_prev = [None]
            z_prev = [None]

            def emit_z(ncb, zaccf):
                z_ps = psB.tile([1, 512], F32, name=f"z_{ncb}", tag="S", bufs=4)
                nc.tensor.matmul(z_ps[:], onesr[:], zaccf[:], start=True, stop=True)
                return z_ps

            def emit_zrec(ncb, z_ps):
                zrec = p2.tile([1, 512], F32, name=f"zrec_{ncb}", tag="zrec", bufs=2)
                nc.vector.reciprocal(out=zrec[:], in_=z_ps[:])
                return zrec

            def emit_pbz(ncb, zrec):
                pbz = psB.tile([128, 512], F32, name=f"pbz_{ncb}", tag="S", bufs=4)
                nc.tensor.matmul(pbz[:], ones32[:], zrec[:], start=True, stop=True)
                return pbz

            def emit_bz(ncb, pbz):
                bz = p2.tile([128, 512], F32, name=f"bz_{ncb}", tag="bz", bufs=2)
                nc.vector.tensor_copy(out=bz[:], in_=pbz[:])
                return bz

            def emit_osb(ncb, ct, u_ps, bz):
                osb = p2.tile([128, 512], F32, name=f"o_{ncb}_{ct}", tag="osb", bufs=4)
                nc.vector.tensor_mul(osb[:], in0=u_ps[:, ct, :], in1=bz[:])
                nc.sync.dma_start(out=o[bass.ts(ct, 128), bass.ts(ncb, 512)], in_=osb[:])

            for ncb in range(NCH):
                er = p2.tile([128, MT, 512], ev_dtype, name=f"er_{ncb}", tag="er", bufs=2)
                zacc4 = p2.tile([128, 4, 512], ev_dtype, name=f"z4_{ncb}", tag="z4", bufs=2)
                u_ps = psB.tile([128, CT, 512], F32, name=f"u_{ncb}", tag="U", bufs=1)
                prev_u, prev_z = u_prev[0], z_prev[0]
                norm_state = {}

                def emit_u(mt):
                    for ct in range(CT):
                        nc.tensor.matmul(u_ps[:, ct, :], vt[:, mt, bass.ts(ct, 128)],
                                         er[:, mt, :], start=(mt == 0), stop=(mt == MT - 1))

                for mt in range(MT):
                    st = psB.tile([128, 512], F32, name=f"st_{ncb}_{mt}", tag="S", bufs=4)
                    for ct in range(CT):
                        nc.tensor.matmul(st[:], k16[:, ct, bass.ts(mt, 128)],
                                         q16[:, ct, bass.ts(ncb, 512)],
                                         start=(ct == 0), stop=(ct == CT - 1))
                    # previous chunk's normalization rides this chunk's stream
                    if prev_u is not None:
                        if mt == 0:
                            norm_state["z"] = emit_z(ncb - 1, prev_z)
                        elif mt == 1:
                            norm_state["zrec"] = emit_zrec(ncb - 1, norm_state["z"])
                        elif mt == 2:
                            norm_state["pbz"] = emit_pbz(ncb - 1, norm_state["zrec"])
                    es = p2.tile([128, 512], F32, name=f"es_{ncb}_{mt}", tag="es", bufs=4)
                    nc.vector.tensor_sub(es[:], in0=st[:], in1=bg[:, bass.ts(ncb, 512)])
                    nc.scalar.activation(out=er[:, mt, :], in_=es[:], func=ACT.Exp)
                    if prev_u is not None:
                        if mt == 2:
                            norm_state["bz"] = emit_bz(ncb - 1, norm_state["pbz"])
                        elif 3 <= mt <= 6:
                            emit_osb(ncb - 1, mt - 3, prev_u, norm_state["bz"])
                    # Z slab accumulation as the slab fills (DVE, off the PE)
                    if mt % 4 == 3:
                        g = mt // 4
                        if g == 0:
                            nc.vector.tensor_copy(out=zacc4[:], in_=er[:, 0:4, :])
                        else:
                            nc.vector.tensor_add(zacc4[:], in0=zacc4[:],
                                                 in1=er[:, bass.ts(g, 4), :])
                    if mt >= depth:
                        emit_u(mt - depth)
                for t in range(MT - depth, MT):
                    emit_u(t)
                zp2 = p2.tile([128, 2, 512], F32, name=f"zp2_{ncb}", tag="zp2", bufs=2)
                nc.vector.tensor_add(zp2[:], in0=zacc4[:, 0:2, :], in1=zacc4[:, 2:4, :])
                zaccf = p2.tile([128, 512], F32, name=f"zf_{ncb}", tag="zf", bufs=2)
                nc.vector.tensor_add(zaccf[:], in0=zp2[:, 0, :], in1=zp2[:, 1, :])
                u_prev[0], z_prev[0] = u_ps, zaccf

            # tail: last chunk's normalization
            z_ps = emit_z(NCH - 1, z_prev[0])
            zrec = emit_zrec(NCH - 1, z_ps)
            pbz = emit_pbz(NCH - 1, zrec)
            bz = emit_bz(NCH - 1, pbz)
            for ct in range(CT):
                emit_osb(NCH - 1, ct, u_prev[0], bz)

    nc.compile()
    return nc


_NC_CACHE = {}


def _get_nc():
    if "nc" not in _NC_CACHE:
        _NC_CACHE["nc"] = build_attention()
    return _NC_CACHE["nc"]


def kernel(content_feat, style_feat, Wq, bq, Wk, bk, Wv, bv):
    content_feat = np.ascontiguousarray(np.asarray(content_feat, dtype=np.float32))
    style_feat = np.ascontiguousarray(np.asarray(style_feat, dtype=np.float32))
    B, C, H, W = content_feat.shape
    N = H * W
    NQ = N // 2
    X = content_feat.reshape(B, C, N)
    Y = style_feat.reshape(B, C, N)
    wqt = np.ascontiguousarray(np.asarray(Wq, dtype=np.float32).T)
    wkt = np.ascontiguousarray(np.asarray(Wk, dtype=np.float32).T)
    wvt = np.ascontiguousarray(np.asarray(Wv, dtype=np.float32).T)
    bq = np.ascontiguousarray(np.asarray(bq, dtype=np.float32))
    bk = np.ascontiguousarray(np.asarray(bk, dtype=np.float32))
    bv = np.ascontiguousarray(np.asarray(bv, dtype=np.float32))

    nc = _get_nc()
    in_maps = []
    for core in range(8):
        b, h = divmod(core, 2)
        if h == 0:
            xqa = X[b]
        else:
            xqa = np.concatenate([X[b][:, NQ:], X[b][:, :NQ]], axis=1)
        in_maps.append({
            "xq": np.ascontiguousarray(xqa), "y": Y[b],
            "wqt": wqt, "wkt": wkt, "wvt": wvt,
            "bq": bq, "bk": bk, "bv": bv,
        })
    res = run_bass_kernel_spmd(nc, in_maps, core_ids=list(range(8)))
    out = np.empty((B, C, N), dtype=np.float32)
    for core in range(8):
        b, h = divmod(core, 2)
        out[b][:, h * NQ:(h + 1) * NQ] = res.results[core]["o"]
    # V bias: softmax rows sum to 1 => Out = (Wv Y)A^T + bv exactly
    out += bv[None, :, None]
    return out.reshape(B, C, H, W)


# revision 24
# speedup vs baseline: 1.0146x; 1.0146x over previous
"""Trainium2 Bass kernel for nn_Attention_50843822850577.

Reference computation (per batch b):
  Q = Wq @ norm(content) + bq ; K = Wk @ norm(style) + bk ; V = Wv @ style + bv
  S = Q^T K  (N x N);  A = softmax(S, axis=-1);  Out = V @ A^T

Sharding: 8 cores = 4 batches x 2 query-halves. Each core gets the full
content/style for its batch (stats need all spatial positions; content is
permuted so the core's query half occupies columns [0, NQ)), computes
Out[:, its-half] and the host scatters halves back together.

v2 structure (phase-1 overlap + PE-offload of softmax bookkeeping):
  - stream Y fully first (V^T matmuls ride the stream), fold K weights,
    then K projection (PE) overlaps the X stream (DMA/DVE only)
  - Q projection, sampled row-max and the G broadcast are emitted per
    512-query chunk so the PE queue never drains
  - E' slab [128, MT, 512] bf16 per chunk: Z = sum E' via DVE slab adds
    (emitted as the slab fills) + one ones-matmul; the old per-m-tile PE
    Z matmuls are gone
  - V tiles and E' in bf16 (validated numerics; fp16 would overflow E')
  - chunk-n normalization (Z matmul, 1/Z, broadcast, U*1/Z, DMA out) is
    interleaved into chunk n+1's S-matmul stream so PE never waits

Numerics (validated in numpy emulation + HW):
  - mean/var stats and all accumulation in fp32
  - normalization folded into the weights: Q = (Wq*inv) @ X_raw + (bq - Wq*inv @ mu)
  - Q/K/S matmuls in fp16 (HW relL2 ~3e-4/matmul)
  - softmax shift G_n = rowmax-over-first-128-keys + 40: the shift cancels
    exactly; sampling margin validated on the reference input distribution
  - E' = exp(S - G) bf16 (range to e^85 ok), V bf16 for the O matmul
  - per-row normalization by Z = sum E' via DVE slab-reduce + ones matmul
"""
import numpy as np

import concourse.bass as bass
import concourse.mybir as mybir
import concourse.tile as tile
from concourse import bacc
from concourse.masks import make_identity
from concourse.bass_utils import run_bass_kernel_spmd

F32 = mybir.dt.float32
F16 = mybir.dt.float16
F32R = mybir.dt.float32r
BF16 = mybir.dt.bfloat16
AX = mybir.AxisListType
ACT = mybir.ActivationFunctionType

EPS = 1e-5
G_OFFSET = 40.0


def build_attention(C=512, NK=4096, NQ=2048, ev_dtype=BF16, hkc=512, depth=5):
    """One-core SPMD program: full attention for one (batch, query-half)."""
    assert C % 128 == 0 and NK % 1024 == 0 and NQ % 512 == 0 and NQ <= NK // 2
    CT = C // 128          # contraction/channel tiles
    MT = NK // 128         # key (m) tiles
    NCH = NQ // 512        # query chunks of 512
    NT = NQ // 128         # query tiles of 128
    HKC = hkc              # n-major streaming chunk width
    NCC = NK // HKC        # streaming chunks per input
    MPC = HKC // 128       # m-tiles per streaming chunk
    ddof_scale = NK / (NK - 1)

    nc = bacc.Bacc("TRN2", target_bir_lowering=False, debug=False)
    xq = nc.dram_tensor("xq", [C, NK], F32, kind="ExternalInput")
    y = nc.dram_tensor("y", [C, NK], F32, kind="ExternalInput")
    wqt = nc.dram_tensor("wqt", [C, C], F32, kind="ExternalInput")
    wkt = nc.dram_tensor("wkt", [C, C], F32, kind="ExternalInput")
    wvt = nc.dram_tensor("wvt", [C, C], F32, kind="ExternalInput")
    bq = nc.dram_tensor("bq", [C], F32, kind="ExternalInput")
    bk = nc.dram_tensor("bk", [C], F32, kind="ExternalInput")
    bv = nc.dram_tensor("bv", [C], F32, kind="ExternalInput")
    o = nc.dram_tensor("o", [C, NQ], F32, kind="ExternalOutput")

    with tile.TileContext(nc) as tc:
      with tc.tile_pool(name="persist", bufs=1) as persist:
        ones32 = persist.tile([1, 128], F32, name="ones32")
        nc.vector.memset(ones32[:], 1.0)
        onesr = persist.tile([128, 1], F32, name="onesr")
        nc.vector.memset(onesr[:], 1.0)
        q16 = persist.tile([128, CT, NQ], F16, name="q16")
        k16 = persist.tile([128, CT, NK], F16, name="k16")
        vt = persist.tile([128, MT, C], ev_dtype, name="vt")
        bg = persist.tile([128, NQ], F32, name="bg")
        ident = persist.tile([128, 128], F32, name="ident")
        make_identity(nc, ident)
        ones16 = persist.tile([1, 128], F16, name="ones16")
        nc.vector.memset(ones16[:], 1.0)

        # ---------------- phase 1: stream + stats + projections ----------
        with (
            tc.tile_pool(name="ph1", bufs=1) as p1,
            tc.tile_pool(name="psA", bufs=3, space="PSUM") as psA,
        ):
            y16 = p1.tile([128, CT, NK], F16, name="y16")
            x16 = p1.tile([128, CT, NQ], F16, name="x16")
            wv16 = p1.tile([128, CT, C], F16, name="wv16")
            wk16 = p1.tile([128, CT, C], F16, name="wk16")
            wq16 = p1.tile([128, CT, C], F16, name="wq16")
            inv_x = p1.tile([128, CT, 1], F32, name="inv_x")
            inv_y = p1.tile([128, CT, 1], F32, name="inv_y")
            mu_x16 = p1.tile([128, CT, 1], F16, name="mu_x16")
            mu_y16 = p1.tile([128, CT, 1], F16, name="mu_y16")
            eps_t = p1.tile([128, 1], F32, name="eps_t")
            nc.vector.memset(eps_t[:], EPS)
            bq_sb = p1.tile([128, CT, 1], F32, name="bq_sb")
            bk_sb = p1.tile([128, CT, 1], F32, name="bk_sb")
            bqp = p1.tile([128, CT, 1], F32, name="bqp")
            bkp = p1.tile([128, CT, 1], F32, name="bkp")
            stats_y = p1.tile([128, CT, 8, 6], F32, name="stats_y")
            stats_x = p1.tile([128, CT, 8, 6], F32, name="stats_x")
            mt_max = p1.tile([128, NT, 1], F32, name="mt_max")
            bgrow = p1.tile([1, NQ], F16, name="bgrow")
            dma_engs = (nc.sync, nc.scalar, nc.gpsimd)

            def fold_stats(stats_t, inv_t, mu16_t, tagp):
                # batched: 4x bn_aggr, then ONE sqrt / reciprocal / copy over
                # [128, CT] so the post-stream serial fold chain is short
                mv4 = p1.tile([128, CT, 2], F32, name=f"mv4_{tagp}", tag="mv")
                for ct in range(CT):
                    nc.vector.bn_aggr(out=mv4[:, ct], in_=stats_t[:, ct])
                std4 = p1.tile([128, CT, 1], F32, name=f"std4_{tagp}", tag="std")
                nc.scalar.activation(out=std4[:], in_=mv4[:, :, 1:2], func=ACT.Sqrt,
                                     bias=eps_t[:], scale=float(ddof_scale))
                nc.vector.reciprocal(out=inv_t[:], in_=std4[:])
                nc.vector.tensor_copy(out=mu16_t[:], in_=mv4[:, :, 0:1])

            def fold_weights(wsrc, wdst, inv_t, eng):
                for ct in range(CT):
                    wraw = p1.tile([128, C], F32, name=f"wraw_{ct}", tag="raw", bufs=2)
                    eng.dma_start(out=wraw[:], in_=wsrc[bass.ts(ct, 128), :])
                    if inv_t is None:
                        nc.vector.tensor_copy(out=wdst[:, ct, :], in_=wraw[:])
                    else:
                        nc.vector.tensor_scalar_mul(wdst[:, ct, :], in0=wraw[:],
                                                    scalar1=inv_t[:, ct, :])

            def fold_bias(wdst, mu16_t, b_sb, bp):
                for ot in range(CT):
                    pb = psA.tile([128, 1], F32, name=f"pb_{ot}", tag="mm")
                    for ct in range(CT):
                        nc.tensor.matmul(pb[:], wdst[:, ct, bass.ts(ot, 128)],
                                         mu16_t[:, ct, :],
                                         start=(ct == 0), stop=(ct == CT - 1))
                    nc.vector.tensor_sub(bp[:, ot, :], in0=b_sb[:, ot, :], in1=pb[:])

            def proj_chunk(w16, src16, bp, dst, j):
                # dst[o, j*512 : +512] = W^T @ src + b  (bias-add on ACT:
                # bias is per output channel = per partition, so the PSUM
                # eviction to fp16 absorbs it and the DVE stays free)
                for ot in range(CT):
                    pq = psA.tile([128, 512], F32, name=f"pq_{ot}_{j}", tag="mm")
                    for ct in range(CT):
                        nc.tensor.matmul(pq[:], w16[:, ct, bass.ts(ot, 128)],
                                         src16[:, ct, bass.ts(j, 512)],
                                         start=(ct == 0), stop=(ct == CT - 1))
                    nc.scalar.activation(out=dst[:, ot, bass.ts(j, 512)], in_=pq[:],
                                         func=ACT.Identity, bias=bp[:, ot, :])

            def stream_chunk(src, raw_tag, stats_t, j):
                raw = p1.tile([128, CT, HKC], F32, name=f"{raw_tag}_{j}",
                              tag=raw_tag, bufs=4 if raw_tag == "rawy" else 2)
                nc.sync.dma_start(
                    out=raw[:],
                    in_=src.rearrange("(t p) n -> p t n", p=128)[:, :, bass.ts(j, HKC)])
                for ct in range(CT):
                    nc.vector.bn_stats(out=stats_t[:, ct, j, :], in_=raw[:, ct, :])
                return raw

            def copy16(raw, dst16, j):
                nc.scalar.copy(out=dst16[:, :, bass.ts(j, HKC)], in_=raw[:])

            def v_chunk(j):
                for mi in range(MPC):
                    mt = j * MPC + mi
                    pv = psA.tile([128, C], F32, name=f"pv_{mt}", tag="mm")
                    for ct in range(CT):
                        nc.tensor.matmul(
                            pv[:], y16[:, ct, bass.ts(mt, 128)], wv16[:, ct, :],
                            start=(ct == 0), stop=(ct == CT - 1))
                    # no +bv here: softmax rows sum to 1, so Out = (Wv Y)A^T + bv
                    # exactly -- bv is added on the host
                    nc.scalar.copy(out=vt[:, mt, :], in_=pv[:])

            # ---- Y stream: stats + fp16 copy, V^T deferred one chunk so the
            # ACT queue emits chunk j+1's copy before chunk j's vt evictions.
            # The last two V chunks are deferred further: they are the PE
            # filler while the Y-stats fold chain (DVE/ACT) runs. ----
            r0 = stream_chunk(y, "rawy", stats_y, 0)
            copy16(r0, y16, 0)
            fold_weights(wvt, wv16, None, nc.gpsimd)
            nc.gpsimd.dma_start(out=bk_sb[:], in_=bk.rearrange("(t p one) -> p t one", p=128, one=1))
            nc.gpsimd.dma_start(out=bq_sb[:], in_=bq.rearrange("(t p one) -> p t one", p=128, one=1))
            for j in range(1, NCC):
                rj = stream_chunk(y, "rawy", stats_y, j)
                copy16(rj, y16, j)
                if j >= 2:
                    v_chunk(j - 2)

            fold_stats(stats_y, inv_y, mu_y16, "y")
            fold_weights(wkt, wk16, inv_y, nc.sync)
            v_chunk(NCC - 2)
            v_chunk(NCC - 1)
            fold_bias(wk16, mu_y16, bk_sb, bkp)

            # ---- X stream (DMA/DVE) interleaved with K projection (PE/ACT).
            # x16 copies are emitted AFTER the K-proj evictions of the same
            # round so they never block the pq PSUM rotation; the last K
            # chunk is the PE filler for the X-stats fold chain. ----
            assert NCC == NK // 512
            for j in range(NCC):
                rxj = stream_chunk(xq, "rawx", stats_x, j)
                if j >= 1:
                    proj_chunk(wk16, y16, bkp, k16, j - 1)
                if j * HKC < NQ:
                    copy16(rxj, x16, j)

            fold_stats(stats_x, inv_x, mu_x16, "x")
            fold_weights(wqt, wq16, inv_x, nc.sync)
            proj_chunk(wk16, y16, bkp, k16, NCC - 1)
            fold_bias(wq16, mu_x16, bq_sb, bqp)

            # ---- Q projection + sampled row-max + G broadcast. Two passes:
            # pss+reduce ride each Q chunk (PE dense), then all transposes,
            # then the per-chunk broadcasts -- no PE<->DVE ping-pong. ----
            for jq in range(NCH):
                proj_chunk(wq16, x16, bqp, q16, jq)
                for nt in range(jq * 4, jq * 4 + 4):
                    pss = psA.tile([128, 128], F32, name=f"pss_{nt}", tag="pss", bufs=2)
                    for ct in range(CT):
                        nc.tensor.matmul(pss[:], q16[:, ct, bass.ts(nt, 128)],
                                         k16[:, ct, 0:128],
                                         start=(ct == 0), stop=(ct == CT - 1))
                    nc.vector.reduce_max(out=mt_max[:, nt, :], in_=pss[:], axis=AX.X)
            for nt in range(NT):
                ps_t = psA.tile([1, 128], F32, name=f"ps_t_{nt}", tag="pst", bufs=2)
                nc.tensor.transpose(ps_t[:], mt_max[:, nt, :], ident[:])
                nc.scalar.activation(out=bgrow[:, bass.ts(nt, 128)], in_=ps_t[:],
                                     func=ACT.Copy, bias=G_OFFSET)
            for jq in range(NCH):
                pbg = psA.tile([128, 512], F32, name=f"pbg_{jq}", tag="mm")
                nc.tensor.matmul(pbg[:], ones16[:], bgrow[:, bass.ts(jq, 512)],
                                 start=True, stop=True)
                nc.vector.tensor_copy(out=bg[:, bass.ts(jq, 512)], in_=pbg[:])

        # ---------------- phase 2: S -> E' -> U, Z -> O -------------------
        with (
            tc.tile_pool(name="ph2", bufs=1) as p2,
            tc.tile_pool(name="psB", bufs=1, space="PSUM") as psB,
        ):
            u_prev = [None]
            z_prev = [None]

            def emit_z(ncb, zaccf):
                z_ps = psB.tile([1, 512], F32, name=f"z_{ncb}", tag="S", bufs=4)
                nc.tensor.matmul(z_ps[:], onesr[:], zaccf[:], start=True, stop=True)
                return z_ps

            def emit_zrec(ncb, z_ps):
                zrec = p2.tile([1, 512], F32, name=f"zrec_{ncb}", tag="zrec", bufs=2)
                nc.vector.reciprocal(out=zrec[:], in_=z_ps[:])
                return zrec

            def emit_pbz(ncb, zrec):
                pbz = psB.tile([128, 512], F32, name=f"pbz_{ncb}", tag="S", bufs=4)
                nc.tensor.matmul(pbz[:], ones32[:], zrec[:], start=True, stop=True)
                return pbz

            def emit_bz(ncb, pbz):
                bz = p2.tile([128, 512], F32, name=f"bz_{ncb}", tag="bz", bufs=2)
                nc.vector.tensor_copy(out=bz[:], in_=pbz[:])
                return bz

            def emit_osb(ncb, ct, u_ps, bz):
                osb = p2.tile([128, 512], F32, name=f"o_{ncb}_{ct}", tag="osb", bufs=4)
                nc.vector.tensor_mul(osb[:], in0=u_ps[:, ct, :], in1=bz[:])
                nc.sync.dma_start(out=o[bass.ts(ct, 128), bass.ts(ncb, 512)], in_=osb[:])

            for ncb in range(NCH):
                er = p2.tile([128, MT, 512], ev_dtype, name=f"er_{ncb}", tag="er", bufs=2)
                zacc4 = p2.tile([128, 4, 512], ev_dtype, name=f"z4_{ncb}", tag="z4", bufs=2)
                u_ps = psB.tile([128, CT, 512], F32, name=f"u_{ncb}", tag="U", bufs=1)
                prev_u, prev_z = u_prev[0], z_prev[0]
                norm_state = {}

                def emit_u(mt):
                    for ct in range(CT):
                        nc.tensor.matmul(u_ps[:, ct, :], vt[:, mt, bass.ts(ct, 128)],
                                         er[:, mt, :], start=(mt == 0), stop=(mt == MT - 1))

                for mt in range(MT):
                    st = psB.tile([128, 512], F32, name=f"st_{ncb}_{mt}", tag="S", bufs=4)
                    for ct in range(CT):
                        nc.tensor.matmul(st[:], k16[:, ct, bass.ts(mt, 128)],
                                         q16[:, ct, bass.ts(ncb, 512)],
                                         start=(ct == 0), stop=(ct == CT - 1))
                    # previous chunk's normalization rides this chunk's stream
                    if prev_u is not None:
                        if mt == 0:
                            norm_state["z"] = emit_z(ncb - 1, prev_z)
                        elif mt == 1:
                            norm_state["zrec"] = emit_zrec(ncb - 1, norm_state["z"])
                        elif mt == 2:
                            norm_state["pbz"] = emit_pbz(ncb - 1, norm_state["zrec"])
                    es = p2.tile([128, 512], F32, name=f"es_{ncb}_{mt}", tag="es", bufs=4)
                    nc.vector.tensor_sub(es[:], in0=st[:], in1=bg[:, bass.ts(ncb, 512)])
                    nc.scalar.activation(out=er[:, mt, :], in_=es[:], func=ACT.Exp)
                    if prev_u is not None:
                        if mt == 2:
                            norm_state["bz"] = emit_bz(ncb - 1, norm_state["pbz"])
                        elif 3 <= mt <= 6:
                            emit_osb(ncb - 1, mt - 3, prev_u, norm_state["bz"])
                    # Z slab accumulation as the slab fills (DVE, off the PE)
                    if mt % 4 == 3:
                        g = mt // 4
                        if g == 0:
                            nc.vector.tensor_copy(out=zacc4[:], in_=er[:, 0:4, :])
                        else:
                            nc.vector.tensor_add(zacc4[:], in0=zacc4[:],
                                                 in1=er[:, bass.ts(g, 4), :])
                    if mt >= depth:
                        emit_u(mt - depth)
                for t in range(MT - depth, MT):
                    emit_u(t)
                zp2 = p2.tile([128, 2, 512], F32, name=f"zp2_{ncb}", tag="zp2", bufs=2)
                nc.vector.tensor_add(zp2[:], in0=zacc4[:, 0:2, :], in1=zacc4[:, 2:4, :])
                zaccf = p2.tile([128, 512], F32, name=f"zf_{ncb}", tag="zf", bufs=2)
                nc.vector.tensor_add(zaccf[:], in0=zp2[:, 0, :], in1=zp2[:, 1, :])
                u_prev[0], z_prev[0] = u_ps, zaccf

            # tail: last chunk's normalization
            z_ps = emit_z(NCH - 1, z_prev[0])
            zrec = emit_zrec(NCH - 1, z_ps)
            pbz = emit_pbz(NCH - 1, zrec)
            bz = emit_bz(NCH - 1, pbz)
            for ct in range(CT):
                emit_osb(NCH - 1, ct, u_prev[0], bz)

    nc.compile()
    return nc


_NC_CACHE = {}


def _get_nc():
    if "nc" not in _NC_CACHE:
        _NC_CACHE["nc"] = build_attention()
    return _NC_CACHE["nc"]


def kernel(content_feat, style_feat, Wq, bq, Wk, bk, Wv, bv):
    content_feat = np.ascontiguousarray(np.asarray(content_feat, dtype=np.float32))
    style_feat = np.ascontiguousarray(np.asarray(style_feat, dtype=np.float32))
    B, C, H, W = content_feat.shape
    N = H * W
    NQ = N // 2
    X = content_feat.reshape(B, C, N)
    Y = style_feat.reshape(B, C, N)
    wqt = np.ascontiguousarray(np.asarray(Wq, dtype=np.float32).T)
    wkt = np.ascontiguousarray(np.asarray(Wk, dtype=np.float32).T)
    wvt = np.ascontiguousarray(np.asarray(Wv, dtype=np.float32).T)
    bq = np.ascontiguousarray(np.asarray(bq, dtype=np.float32))
    bk = np.ascontiguousarray(np.asarray(bk, dtype=np.float32))
    bv = np.ascontiguousarray(np.asarray(bv, dtype=np.float32))

    nc = _get_nc()
    in_maps = []
    for core in range(8):
        b, h = divmod(core, 2)
        if h == 0:
            xqa = X[b]
        else:
            xqa = np.concatenate([X[b][:, NQ:], X[b][:, :NQ]], axis=1)
        in_maps.append({
            "xq": np.ascontiguousarray(xqa), "y": Y[b],
            "wqt": wqt, "wkt": wkt, "wvt": wvt,
            "bq": bq, "bk": bk, "bv": bv,
        })
    res = run_bass_kernel_spmd(nc, in_maps, core_ids=list(range(8)))
    out = np.empty((B, C, N), dtype=np.float32)
    for core in range(8):
        b, h = divmod(core, 2)
        out[b][:, h * NQ:(h + 1) * NQ] = res.results[core]["o"]
    # V bias: softmax rows sum to 1 => Out = (Wv Y)A^T + bv exactly
    out += bv[None, :, None]
    return out.reshape(B, C, H, W)


# revision 25
# speedup vs baseline: 1.1893x; 1.1722x over previous
"""Trainium2 Bass kernel for nn_Attention_50843822850577.

Reference computation (per batch b):
  Q = Wq @ norm(content) + bq ; K = Wk @ norm(style) + bk ; V = Wv @ style + bv
  S = Q^T K  (N x N);  A = softmax(S, axis=-1);  Out = V @ A^T

Sharding: 8 cores = 4 batches x 2 query-halves. Each core gets the full
content/style for its batch (stats need all spatial positions; content is
permuted so the core's query half occupies columns [0, NQ)), computes
Out[:, its-half] and the host scatters halves back together.

v2 structure (phase-1 overlap + PE-offload of softmax bookkeeping):
  - stream Y fully first (V^T matmuls ride the stream), fold K weights,
    then K projection (PE) overlaps the X stream (DMA/DVE only)
  - Q projection, sampled row-max and the G broadcast are emitted per
    512-query chunk so the PE queue never drains
  - E' slab [128, MT, 512] bf16 per chunk: Z = sum E' via DVE slab adds
    (emitted as the slab fills) + one ones-matmul; the old per-m-tile PE
    Z matmuls are gone
  - V tiles and E' in bf16 (validated numerics; fp16 would overflow E')
  - chunk-n normalization (Z matmul, 1/Z, broadcast, U*1/Z, DMA out) is
    interleaved into chunk n+1's S-matmul stream so PE never waits

Numerics (validated in numpy emulation + HW):
  - mean/var stats and all accumulation in fp32
  - normalization folded into the weights: Q = (Wq*inv) @ X_raw + (bq - Wq*inv @ mu)
  - Q/K/S matmuls in fp16 (HW relL2 ~3e-4/matmul)
  - softmax shift G_n = rowmax-over-first-128-keys + 40: the shift cancels
    exactly; sampling margin validated on the reference input distribution
  - E' = exp(S - G) bf16 (range to e^85 ok), V bf16 for the O matmul
  - per-row normalization by Z = sum E' via DVE slab-reduce + ones matmul
"""
import numpy as np

import concourse.bass as bass
import concourse.mybir as mybir
import concourse.tile as tile
from concourse import bacc
from concourse.masks import make_identity
from concourse.bass_utils import run_bass_kernel_spmd

F32 = mybir.dt.float32
F16 = mybir.dt.float16
F32R = mybir.dt.float32r
BF16 = mybir.dt.bfloat16
AX = mybir.AxisListType
ACT = mybir.ActivationFunctionType

EPS = 1e-5
G_OFFSET = 40.0


def build_attention(C=512, NK=4096, NQ=2048, ev_dtype=BF16, hkc=512, depth=5):
    """One-core SPMD program: full attention for one (batch, query-half)."""
    assert C % 128 == 0 and NK % 1024 == 0 and NQ % 512 == 0 and NQ <= NK // 2
    CT = C // 128          # contraction/channel tiles
    MT = NK // 128         # key (m) tiles
    NCH = NQ // 512        # query chunks of 512
    NT = NQ // 128         # query tiles of 128
    HKC = hkc              # n-major streaming chunk width
    NCC = NK // HKC        # streaming chunks per input
    MPC = HKC // 128       # m-tiles per streaming chunk
    ddof_scale = NK / (NK - 1)

    nc = bacc.Bacc("TRN2", target_bir_lowering=False, debug=False)
    xq = nc.dram_tensor("xq", [C, NK], F32, kind="ExternalInput")
    y = nc.dram_tensor("y", [C, NK], F32, kind="ExternalInput")
    wqt = nc.dram_tensor("wqt", [C, C], F32, kind="ExternalInput")
    wkt = nc.dram_tensor("wkt", [C, C], F32, kind="ExternalInput")
    wvt = nc.dram_tensor("wvt", [C, C], F32, kind="ExternalInput")
    bq = nc.dram_tensor("bq", [C], F32, kind="ExternalInput")
    bk = nc.dram_tensor("bk", [C], F32, kind="ExternalInput")
    bv = nc.dram_tensor("bv", [C], F32, kind="ExternalInput")
    o = nc.dram_tensor("o", [C, NQ], F32, kind="ExternalOutput")

    with tile.TileContext(nc) as tc:
      with tc.tile_pool(name="persist", bufs=1) as persist:
        ones32 = persist.tile([1, 128], F32, name="ones32")
        nc.vector.memset(ones32[:], 1.0)
        onesr = persist.tile([128, 1], F32, name="onesr")
        nc.vector.memset(onesr[:], 1.0)
        q16 = persist.tile([128, CT, NQ], F16, name="q16")
        k16 = persist.tile([128, CT, NK], F16, name="k16")
        vt = persist.tile([128, MT, C], ev_dtype, name="vt")
        bg = persist.tile([128, NQ], F32, name="bg")
        ident = persist.tile([128, 128], F32, name="ident")
        make_identity(nc, ident)
        ones16 = persist.tile([1, 128], F16, name="ones16")
        nc.vector.memset(ones16[:], 1.0)

        # ---------------- phase 1: stream + stats + projections ----------
        with (
            tc.tile_pool(name="ph1", bufs=1) as p1,
            tc.tile_pool(name="psA", bufs=3, space="PSUM") as psA,
        ):
            y16 = p1.tile([128, CT, NK], F16, name="y16")
            x16 = p1.tile([128, CT, NQ], F16, name="x16")
            wv16 = p1.tile([128, CT, C], F16, name="wv16")
            wk16 = p1.tile([128, CT, C], F16, name="wk16")
            wq16 = p1.tile([128, CT, C], F16, name="wq16")
            inv_x = p1.tile([128, CT, 1], F32, name="inv_x")
            inv_y = p1.tile([128, CT, 1], F32, name="inv_y")
            mu_x16 = p1.tile([128, CT, 1], F16, name="mu_x16")
            mu_y16 = p1.tile([128, CT, 1], F16, name="mu_y16")
            eps_t = p1.tile([128, 1], F32, name="eps_t")
            nc.vector.memset(eps_t[:], EPS)
            bq_sb = p1.tile([128, CT, 1], F32, name="bq_sb")
            bk_sb = p1.tile([128, CT, 1], F32, name="bk_sb")
            bqp = p1.tile([128, CT, 1], F32, name="bqp")
            bkp = p1.tile([128, CT, 1], F32, name="bkp")
            stats_y = p1.tile([128, CT, 8, 6], F32, name="stats_y")
            stats_x = p1.tile([128, CT, 8, 6], F32, name="stats_x")
            mt_max = p1.tile([128, NT, 1], F32, name="mt_max")
            bgrow = p1.tile([1, NQ], F16, name="bgrow")
            dma_engs = (nc.sync, nc.scalar, nc.gpsimd)

            def fold_stats(stats_t, inv_t, mu16_t, tagp):
                # batched: 4x bn_aggr, then ONE sqrt / reciprocal / copy over
                # [128, CT] so the post-stream serial fold chain is short
                mv4 = p1.tile([128, CT, 2], F32, name=f"mv4_{tagp}", tag="mv")
                for ct in range(CT):
                    nc.vector.bn_aggr(out=mv4[:, ct], in_=stats_t[:, ct])
                std4 = p1.tile([128, CT, 1], F32, name=f"std4_{tagp}", tag="std")
                nc.scalar.activation(out=std4[:], in_=mv4[:, :, 1:2], func=ACT.Sqrt,
                                     bias=eps_t[:], scale=float(ddof_scale))
                nc.vector.reciprocal(out=inv_t[:], in_=std4[:])
                nc.vector.tensor_copy(out=mu16_t[:], in_=mv4[:, :, 0:1])

            def fold_weights(wsrc, wdst, inv_t, eng):
                for ct in range(CT):
                    wraw = p1.tile([128, C], F32, name=f"wraw_{ct}", tag="raw", bufs=2)
                    eng.dma_start(out=wraw[:], in_=wsrc[bass.ts(ct, 128), :])
                    if inv_t is None:
                        nc.vector.tensor_copy(out=wdst[:, ct, :], in_=wraw[:])
                    else:
                        nc.vector.tensor_scalar_mul(wdst[:, ct, :], in0=wraw[:],
                                                    scalar1=inv_t[:, ct, :])

            def fold_bias(wdst, mu16_t, b_sb, bp):
                for ot in range(CT):
                    pb = psA.tile([128, 1], F32, name=f"pb_{ot}", tag="mm")
                    for ct in range(CT):
                        nc.tensor.matmul(pb[:], wdst[:, ct, bass.ts(ot, 128)],
                                         mu16_t[:, ct, :],
                                         start=(ct == 0), stop=(ct == CT - 1))
                    nc.vector.tensor_sub(bp[:, ot, :], in0=b_sb[:, ot, :], in1=pb[:])

            def proj_chunk(w16, src16, bp, dst, j):
                # dst[o, j*512 : +512] = W^T @ src + b  (bias-add on ACT:
                # bias is per output channel = per partition, so the PSUM
                # eviction to fp16 absorbs it and the DVE stays free)
                for ot in range(CT):
                    pq = psA.tile([128, 512], F32, name=f"pq_{ot}_{j}", tag="mm")
                    for ct in range(CT):
                        nc.tensor.matmul(pq[:], w16[:, ct, bass.ts(ot, 128)],
                                         src16[:, ct, bass.ts(j, 512)],
                                         start=(ct == 0), stop=(ct == CT - 1))
                    nc.scalar.activation(out=dst[:, ot, bass.ts(j, 512)], in_=pq[:],
                                         func=ACT.Identity, bias=bp[:, ot, :])

            def stream_chunk(src, raw_tag, stats_t, j):
                raw = p1.tile([128, CT, HKC], F32, name=f"{raw_tag}_{j}",
                              tag=raw_tag, bufs=4 if raw_tag == "rawy" else 2)
                nc.sync.dma_start(
                    out=raw[:],
                    in_=src.rearrange("(t p) n -> p t n", p=128)[:, :, bass.ts(j, HKC)])
                for ct in range(CT):
                    nc.vector.bn_stats(out=stats_t[:, ct, j, :], in_=raw[:, ct, :])
                return raw

            def copy16(raw, dst16, j):
                nc.scalar.copy(out=dst16[:, :, bass.ts(j, HKC)], in_=raw[:])

            def v_chunk(j):
                for mi in range(MPC):
                    mt = j * MPC + mi
                    pv = psA.tile([128, C], F32, name=f"pv_{mt}", tag="mm")
                    for ct in range(CT):
                        nc.tensor.matmul(
                            pv[:], y16[:, ct, bass.ts(mt, 128)], wv16[:, ct, :],
                            start=(ct == 0), stop=(ct == CT - 1))
                    # no +bv here: softmax rows sum to 1, so Out = (Wv Y)A^T + bv
                    # exactly -- bv is added on the host
                    nc.scalar.copy(out=vt[:, mt, :], in_=pv[:])

            # ---- Y stream: stats + fp16 copy, V^T deferred one chunk so the
            # ACT queue emits chunk j+1's copy before chunk j's vt evictions.
            # The last two V chunks are deferred further: they are the PE
            # filler while the Y-stats fold chain (DVE/ACT) runs. ----
            r0 = stream_chunk(y, "rawy", stats_y, 0)
            copy16(r0, y16, 0)
            fold_weights(wvt, wv16, None, nc.gpsimd)
            nc.gpsimd.dma_start(out=bk_sb[:], in_=bk.rearrange("(t p one) -> p t one", p=128, one=1))
            nc.gpsimd.dma_start(out=bq_sb[:], in_=bq.rearrange("(t p one) -> p t one", p=128, one=1))
            for j in range(1, NCC):
                rj = stream_chunk(y, "rawy", stats_y, j)
                copy16(rj, y16, j)
                if j >= 2:
                    v_chunk(j - 2)

            fold_stats(stats_y, inv_y, mu_y16, "y")
            fold_weights(wkt, wk16, inv_y, nc.sync)
            v_chunk(NCC - 2)
            v_chunk(NCC - 1)
            fold_bias(wk16, mu_y16, bk_sb, bkp)

            # ---- X stream (DMA/DVE) interleaved with K projection (PE/ACT).
            # x16 copies are emitted AFTER the K-proj evictions of the same
            # round so they never block the pq PSUM rotation; the last K
            # chunk is the PE filler for the X-stats fold chain. ----
            assert NCC == NK // 512
            for j in range(NCC):
                rxj = stream_chunk(xq, "rawx", stats_x, j)
                if j >= 1:
                    proj_chunk(wk16, y16, bkp, k16, j - 1)
                if j * HKC < NQ:
                    copy16(rxj, x16, j)

            fold_stats(stats_x, inv_x, mu_x16, "x")
            fold_weights(wqt, wq16, inv_x, nc.sync)
            proj_chunk(wk16, y16, bkp, k16, NCC - 1)
            fold_bias(wq16, mu_x16, bq_sb, bqp)

            # ---- Q projection + sampled row-max + G broadcast. Two passes:
            # pss+reduce ride each Q chunk (PE dense), then all transposes,
            # then the per-chunk broadcasts -- no PE<->DVE ping-pong. ----
            for jq in range(NCH):
                proj_chunk(wq16, x16, bqp, q16, jq)
                for nt in range(jq * 4, jq * 4 + 4):
                    pss = psA.tile([128, 128], F32, name=f"pss_{nt}", tag="pss", bufs=2)
                    for ct in range(CT):
                        nc.tensor.matmul(pss[:], q16[:, ct, bass.ts(nt, 128)],
                                         k16[:, ct, 0:128],
                                         start=(ct == 0), stop=(ct == CT - 1))
                    nc.vector.reduce_max(out=mt_max[:, nt, :], in_=pss[:], axis=AX.X)
            for nt in range(NT):
                ps_t = psA.tile([1, 128], F32, name=f"ps_t_{nt}", tag="pst", bufs=2)
                nc.tensor.transpose(ps_t[:], mt_max[:, nt, :], ident[:])
                nc.scalar.activation(out=bgrow[:, bass.ts(nt, 128)], in_=ps_t[:],
                                     func=ACT.Copy, bias=G_OFFSET)
            for jq in range(NCH):
                pbg = psA.tile([128, 512], F32, name=f"pbg_{jq}", tag="mm")
                nc.tensor.matmul(pbg[:], ones16[:], bgrow[:, bass.ts(jq, 512)],
                                 start=True, stop=True)
                nc.vector.tensor_copy(out=bg[:, bass.ts(jq, 512)], in_=pbg[:])

        # ---------------- phase 2: S -> E' -> U, Z -> O -------------------
        with (
            tc.tile_pool(name="ph2", bufs=1) as p2,
            tc.tile_pool(name="psB", bufs=1, space="PSUM") as psB,
        ):
            u_prev = [None]
            z_prev = [None]

            def emit_z(ncb, zaccf):
                z_ps = psB.tile([1, 512], F32, name=f"z_{ncb}", tag="Z", bufs=1)
                nc.tensor.matmul(z_ps[:], onesr[:], zaccf[:], start=True, stop=True)
                return z_ps

            def emit_zrec(ncb, z_ps):
                zrec = p2.tile([1, 512], F32, name=f"zrec_{ncb}", tag="zrec", bufs=2)
                nc.vector.reciprocal(out=zrec[:], in_=z_ps[:])
                return zrec

            def emit_pbz(ncb, zrec):
                pbz = psB.tile([128, 512], F32, name=f"pbz_{ncb}", tag="S", bufs=3)
                nc.tensor.matmul(pbz[:], ones32[:], zrec[:], start=True, stop=True)
                return pbz

            def emit_bz(ncb, pbz):
                bz = p2.tile([128, 512], F32, name=f"bz_{ncb}", tag="bz", bufs=2)
                nc.vector.tensor_copy(out=bz[:], in_=pbz[:])
                return bz

            def emit_osb(ncb, ct, u_ps, bz):
                osb = p2.tile([128, 512], F32, name=f"o_{ncb}_{ct}", tag="osb", bufs=4)
                nc.vector.tensor_mul(osb[:], in0=u_ps[:, ct, :], in1=bz[:])
                nc.sync.dma_start(out=o[bass.ts(ct, 128), bass.ts(ncb, 512)], in_=osb[:])

            for ncb in range(NCH):
                er = p2.tile([128, MT, 512], ev_dtype, name=f"er_{ncb}", tag="er", bufs=2)
                zacc4 = p2.tile([128, 4, 512], ev_dtype, name=f"z4_{ncb}", tag="z4", bufs=2)
                u_ps = psB.tile([128, CT, 512], F32, name=f"u_{ncb}", tag="U", bufs=1)
                prev_u, prev_z = u_prev[0], z_prev[0]
                norm_state = {}

                def emit_u(mt):
                    for ct in range(CT):
                        nc.tensor.matmul(u_ps[:, ct, :], vt[:, mt, bass.ts(ct, 128)],
                                         er[:, mt, :], start=(mt == 0), stop=(mt == MT - 1))

                for mt in range(MT):
                    st = psB.tile([128, 512], F32, name=f"st_{ncb}_{mt}", tag="S", bufs=3)
                    for ct in range(CT):
                        nc.tensor.matmul(st[:], k16[:, ct, bass.ts(mt, 128)],
                                         q16[:, ct, bass.ts(ncb, 512)],
                                         start=(ct == 0), stop=(ct == CT - 1))
                    # previous chunk's normalization rides this chunk's stream
                    if prev_u is not None:
                        if mt == 0:
                            norm_state["z"] = emit_z(ncb - 1, prev_z)
                        elif mt == 1:
                            norm_state["zrec"] = emit_zrec(ncb - 1, norm_state["z"])
                        elif mt == 2:
                            norm_state["pbz"] = emit_pbz(ncb - 1, norm_state["zrec"])
                    es = p2.tile([128, 512], F32, name=f"es_{ncb}_{mt}", tag="es", bufs=4)
                    nc.vector.tensor_sub(es[:], in0=st[:], in1=bg[:, bass.ts(ncb, 512)])
                    nc.scalar.activation(out=er[:, mt, :], in_=es[:], func=ACT.Exp)
                    if prev_u is not None:
                        if mt == 2:
                            norm_state["bz"] = emit_bz(ncb - 1, norm_state["pbz"])
                        elif 3 <= mt <= 6:
                            emit_osb(ncb - 1, mt - 3, prev_u, norm_state["bz"])
                    # Z slab accumulation as the slab fills (DVE, off the PE)
                    if mt % 4 == 3:
                        g = mt // 4
                        if g == 0:
                            nc.vector.tensor_copy(out=zacc4[:], in_=er[:, 0:4, :])
                        else:
                            nc.vector.tensor_add(zacc4[:], in0=zacc4[:],
                                                 in1=er[:, bass.ts(g, 4), :])
                    if mt >= depth:
                        emit_u(mt - depth)
                for t in range(MT - depth, MT):
                    emit_u(t)
                zp2 = p2.tile([128, 2, 512], F32, name=f"zp2_{ncb}", tag="zp2", bufs=2)
                nc.vector.tensor_add(zp2[:], in0=zacc4[:, 0:2, :], in1=zacc4[:, 2:4, :])
                zaccf = p2.tile([128, 512], F32, name=f"zf_{ncb}", tag="zf", bufs=2)
                nc.vector.tensor_add(zaccf[:], in0=zp2[:, 0, :], in1=zp2[:, 1, :])
                u_prev[0], z_prev[0] = u_ps, zaccf

            # tail: last chunk's normalization
            z_ps = emit_z(NCH - 1, z_prev[0])
            zrec = emit_zrec(NCH - 1, z_ps)
            pbz = emit_pbz(NCH - 1, zrec)
            bz = emit_bz(NCH - 1, pbz)
            for ct in range(CT):
                emit_osb(NCH - 1, ct, u_prev[0], bz)

    nc.compile()
    return nc


_NC_CACHE = {}


def _get_nc():
    if "nc" not in _NC_CACHE:
        _NC_CACHE["nc"] = build_attention()
    return _NC_CACHE["nc"]


def kernel(content_feat, style_feat, Wq, bq, Wk, bk, Wv, bv):
    content_feat = np.ascontiguousarray(np.asarray(content_feat, dtype=np.float32))
    style_feat = np.ascontiguousarray(np.asarray(style_feat, dtype=np.float32))
    B, C, H, W = content_feat.shape
    N = H * W
    NQ = N // 2
    X = content_feat.reshape(B, C, N)
    Y = style_feat.reshape(B, C, N)
    wqt = np.ascontiguousarray(np.asarray(Wq, dtype=np.float32).T)
    wkt = np.ascontiguousarray(np.asarray(Wk, dtype=np.float32).T)
    wvt = np.ascontiguousarray(np.asarray(Wv, dtype=np.float32).T)
    bq = np.ascontiguousarray(np.asarray(bq, dtype=np.float32))
    bk = np.ascontiguousarray(np.asarray(bk, dtype=np.float32))
    bv = np.ascontiguousarray(np.asarray(bv, dtype=np.float32))

    nc = _get_nc()
    in_maps = []
    for core in range(8):
        b, h = divmod(core, 2)
        if h == 0:
            xqa = X[b]
        else:
            xqa = np.concatenate([X[b][:, NQ:], X[b][:, :NQ]], axis=1)
        in_maps.append({
            "xq": np.ascontiguousarray(xqa), "y": Y[b],
            "wqt": wqt, "wkt": wkt, "wvt": wvt,
            "bq": bq, "bk": bk, "bv": bv,
        })
    res = run_bass_kernel_spmd(nc, in_maps, core_ids=list(range(8)))
    out = np.empty((B, C, N), dtype=np.float32)
    for core in range(8):
        b, h = divmod(core, 2)
        out[b][:, h * NQ:(h + 1) * NQ] = res.results[core]["o"]
    # V bias: softmax rows sum to 1 => Out = (Wv Y)A^T + bv exactly
    out += bv[None, :, None]
    return out.reshape(B, C, H, W)
